# revision 33
# baseline (speedup 1.0000x reference)
"""MoE top-2 dispatch -> per-expert Linear -> gated combine, on 8 TRN2 cores.

Single fused NEFF, data-parallel over tokens, transposed compute:

Host side does dispatch bookkeeping only (zero FLOPs): tokens are typed by
their expert pair (a, b) with types ordered by combine-ready time (b, a);
each type is round-robined across the 8 cores and padded to a common block
size K_t so one SPMD program serves every core.  The routed activations are
gathered per expert segment in d-blocked transposed layout [ki, ko, col],
and gate values are replicated to 128 partitions host-side.

Device: per expert segment, W_e is the PE-stationary operand and the
gathered x columns stream through, accumulating into 4-bank PSUM tiles
(8 o-blocks, double buffered).  DVE evicts PSUM with the per-column gate
multiply into two fp16 arenas (first/second expert roles, static free-axis
offsets).  After each segment, the newly-ready pair blocks are combined
(arena1 + arena2 -> fp32) and DMA'd out in 512-column groups, so the
combine and output DMA fully overlap the remaining matmuls.  The output is
written transposed [128, 8, n_slots]; the host un-transposes and scatters
slots back to token order (pure indexing).

Self-contained: shapes hardcoded for B=16384, E=8, D=1024, O=1024, K=2.
"""

import os
import sys
import types

sys.path.insert(0, "/opt/trn_rl_repo")

import ml_dtypes
import numpy as np

import concourse.bass as bass
import concourse.mybir as mybir
from concourse import bass_utils
from concourse.tile import TileContext

B, E, D, O = 16384, 8, 1024, 1024
N_CORES = 8
P = 128
KO = D // P  # contraction chunks
OB = O // P  # output 128-blocks
CHUNK = 512  # max psum columns per accumulation (one fp32 bank)

# Types ordered by combine-ready time: type (a, b) is ready after segment b.
TYPES = [(a, b) for b in range(1, E) for a in range(b)]
NT = len(TYPES)

MAX_WAITS = int(os.environ.get("MOE_MAX_WAITS", "1"))


def _patch_tile_drain():
    """Public-walrus workaround: walrus codegen rejects instructions carrying
    more than a couple of sync-wait commands.  Tile's add_semaphores can put
    several waits on one instruction (and the kernel-tail drain carries one
    per live processor).  Hoist excess waits onto single-wait nop carriers
    emitted just before the instruction on the same engine."""
    from concourse.tile import TileContext as TC
    from concourse.vector_clock import ScopedClock

    if getattr(TC, "_moe_drain_patched", False):
        return

    orig_add = TC._add_instruction

    def _add_instruction(self, inst):
        si = getattr(inst, "sync_info", None)
        waits = list(si.on_wait or []) if si is not None else []
        if len(waits) > MAX_WAITS:
            hoist = waits[: len(waits) - MAX_WAITS]
            keep = waits[len(waits) - MAX_WAITS :]
            for w in hoist:
                nop = mybir.InstNoOp(
                    name=self.nc.get_next_instruction_name(),
                    engine=inst.engine,
                    bass_nofuse=True,
                    sync_info=mybir.SyncInfo(on_wait=[w], on_update=[]),
                )
                orig_add(self, nop)
            inst.sync_info = mybir.SyncInfo(
                on_wait=keep, on_update=list(si.on_update or [])
            )
        orig_add(self, inst)

    def _drain_and_barrier(self, tick_clock, wait_clock):
        carrier = self.nc.sync.nop(nofuse=True)
        wait_clock.add_sem_waits(
            carrier.ins, ScopedClock({None: tick_clock.global_clock})
        )
        si = carrier.ins.sync_info
        waits = list(si.on_wait or []) if si is not None else []
        if len(waits) > 1:
            carrier.ins.sync_info = mybir.SyncInfo(
                on_wait=waits[:1], on_update=list(si.on_update or [])
            )
            for w in waits[1:]:
                extra = self.nc.sync.nop(nofuse=True)
                extra.ins.sync_info = mybir.SyncInfo(on_wait=[w], on_update=[])
        self.nc.sync.drain()
        self.nc.all_engine_barrier()
        assert self.sems is not None
        popped = self.nc._tile_sem_poison_stack.pop()
        assert popped is self._sem_poison
        self.nc.clear_and_free_semaphores(list(self.sems.allocated().values()))
        self.nc.all_engine_barrier()

    TC._add_instruction = _add_instruction
    TC._drain_and_barrier = _drain_and_barrier
    TC._moe_drain_patched = True


class Plan:
    """Global (gates-derived) layout shared by all cores."""

    def __init__(self, gates):
        exp = np.argsort(-gates, axis=1)[:, :2]
        e1 = np.minimum(exp[:, 0], exp[:, 1])
        e2 = np.maximum(exp[:, 0], exp[:, 1])
        tcode = e1 * E + e2
        self.toks_t = [
            np.nonzero(tcode == a * E + b)[0].astype(np.int64) for (a, b) in TYPES
        ]
        self.K = [
            int(np.ceil(len(tk) / N_CORES)) for tk in self.toks_t
        ]  # common per-core block size
        self.out_off = np.concatenate([[0], np.cumsum(self.K)]).astype(np.int64)
        self.n_slots = int(self.out_off[-1])
        # segment structure: blocks of expert e in TYPES order
        self.blocks = [
            [t for t in range(NT) if e in TYPES[t]] for e in range(E)
        ]
        self.S = [sum(self.K[t] for t in bl) for bl in self.blocks]
        self.seg_base = np.concatenate([[0], np.cumsum(self.S)]).astype(np.int64)
        self.PAIRS = int(self.seg_base[-1])
        assert self.PAIRS == 2 * self.n_slots
        # eviction runs per segment: (seg_off_local, out_off, len, role)
        # role 1: e is first expert of type -> arena1; role 2 -> arena2.
        self.runs = []
        for e in range(E):
            rr = []
            off = 0
            for t in self.blocks[e]:
                k = self.K[t]
                if k == 0:
                    continue
                role = 1 if TYPES[t][0] == e else 2
                oo = int(self.out_off[t])
                if rr and rr[-1][3] == role and rr[-1][1] + rr[-1][2] == oo:
                    rr[-1] = (rr[-1][0], rr[-1][1], rr[-1][2] + k, role)
                else:
                    rr.append((off, oo, k, role))
                off += k
            self.runs.append(rr)
        # after segment e, newly combine-ready out cols are
        # [ready_lo[e], ready_hi[e]) == the types with b == e
        self.ready_lo = [int(self.out_off[e * (e - 1) // 2]) for e in range(E)]
        self.ready_hi = [int(self.out_off[e * (e + 1) // 2]) for e in range(E)]
        assert self.ready_hi[E - 1] == self.n_slots
        # combine/flush pieces: per (segment, chunk), the newly completed
        # role-2 out cols.  Pieces tile [0, n_slots) in order.
        self.seg_chunks = [_chunks(s) for s in self.S]
        self.n2len = [self.ready_hi[e] - self.ready_lo[e] for e in range(E)]
        self.pieces = []  # (e, chunk_idx, r0, r1), each <= 256 cols so the
        # final add+store chains pipeline instead of serializing
        for e in range(E):
            for ci, (c0, L) in enumerate(self.seg_chunks[e]):
                r0 = self.ready_lo[e] + min(c0, self.n2len[e])
                r1 = self.ready_lo[e] + min(c0 + L, self.n2len[e])
                if r1 > r0:
                    nsub = -(-(r1 - r0) // 256)
                    bounds = np.linspace(r0, r1, nsub + 1).astype(int)
                    for s0, s1 in zip(bounds[:-1], bounds[1:]):
                        self.pieces.append((e, ci, int(s0), int(s1)))
        assert self.pieces and self.pieces[0][2] == 0
        assert all(
            p[2] == q[3] for p, q in zip(self.pieces[1:], self.pieces[:-1])
        )
        assert self.pieces[-1][3] == self.n_slots

    def core_tokens(self, c):
        """Per-type token lists for core c (each len <= K[t])."""
        return [tk[c::N_CORES] for tk in self.toks_t]


def _build_core_inputs(x, gates, plan, c, np_dt):
    toks = plan.core_tokens(c)
    # padded slot->token per type (pads use token 0 with gate 0)
    slot_tok = []
    for t in range(NT):
        arr = np.zeros(plan.K[t], np.int64)
        arr[: len(toks[t])] = toks[t]
        slot_tok.append(arr)
    # gathered x, chunk-major flat layout: chunk at seg col c0 occupies flat
    # cols [KO*(base+c0), KO*(base+c0+L)) as [KO, L] (contiguous/partition).
    xg = np.empty((P, KO * plan.PAIRS), np_dt)
    g_flat = np.zeros(plan.PAIRS, np.float32)
    for e in range(E):
        idx = []
        gv = []
        for t in plan.blocks[e]:
            st = slot_tok[t]
            idx.append(st)
            gvals = np.zeros(plan.K[t], np.float32)
            gvals[: len(toks[t])] = gates[toks[t], e]
            gv.append(gvals)
        idx = np.concatenate(idx) if idx else np.zeros(0, np.int64)
        base = int(plan.seg_base[e])
        for (c0, L) in plan.seg_chunks[e]:
            xs = x[idx[c0 : c0 + L]].astype(np_dt)  # [L, D]
            blk = xs.reshape(L, KO, P).transpose(2, 1, 0)  # [P, KO, L]
            f0 = KO * (base + c0)
            xg[:, f0 : f0 + KO * L] = blk.reshape(P, KO * L)
        g_flat[base : base + plan.S[e]] = np.concatenate(gv)
    g_rep = np.ascontiguousarray(
        np.broadcast_to(g_flat[None, :], (P, plan.PAIRS))
    ).astype(np.float16)
    return {"xg": np.ascontiguousarray(xg), "g": g_rep}


def _chunks(S):
    """Split S columns into balanced chunks of <= CHUNK."""
    n = max(1, -(-S // CHUNK))
    base = S // n
    rem = S % n
    out = []
    c0 = 0
    for i in range(n):
        ln = base + (1 if i < rem else 0)
        out.append((c0, ln))
        c0 += ln
    return out


def _build_program(plan, dt, adt):
    """All DRAM layouts are contiguous per partition for each DMA issued, so
    every dma_start lowers to ~128 descriptors instead of ~1024 (descriptor
    generation on the issuing engine was the start-latency bottleneck)."""
    nc = bass.Bass(target_bir_lowering=False, trn_type="TRN2")
    xg_d = nc.dram_tensor(
        "xg", [P, KO * plan.PAIRS], dt, kind="ExternalInput"
    )
    w_d = nc.dram_tensor(
        "w", [E, 2, P, KO, O // 2], dt, kind="ExternalInput"
    )
    g_d = nc.dram_tensor("g", [P, plan.PAIRS], adt, kind="ExternalInput")
    out_d = nc.dram_tensor(
        "out", [P, OB * plan.n_slots], mybir.dt.float32, kind="ExternalOutput"
    )
    seg_chunks = plan.seg_chunks
    # segment cols [0, n2len[e]) are the role-2 blocks (they sort first);
    # seg col i < n2len maps 1:1 to out col ready_lo[e] + i.
    n2len = plan.n2len

    with TileContext(nc) as tc:
        with (
            tc.tile_pool(name="const", bufs=1) as cpool,
            tc.tile_pool(name="wp", bufs=4) as wpool,
            tc.tile_pool(name="xp", bufs=6) as xpool,
            tc.tile_pool(name="ar", bufs=1) as apool,
            tc.tile_pool(name="og", bufs=2) as ogpool,
            tc.tile_pool(name="ps", bufs=4, space="PSUM") as pspool,
        ):
            arena1 = apool.tile([P, OB, plan.n_slots], adt)
            arena2 = apool.tile([P, OB, plan.n_slots], adt)

            # PE warm-up: junk matmuls burn the 1.2GHz activity-ramp window
            # while the first input DMAs are in flight.
            warm_w = cpool.tile([1, P], dt)
            warm_x = cpool.tile([1, CHUNK], dt)
            nc.vector.memset(warm_w[:], 0.0)
            nc.vector.memset(warm_x[:], 0.0)
            wps = pspool.tile([P, 2, CHUNK], mybir.dt.float32, tag="ps", name="wps")
            for _ in range(10):
                nc.tensor.matmul(
                    out=wps[:, 0, :],
                    lhsT=warm_w[:1, :],
                    rhs=warm_x[:1, :],
                    start=True,
                    stop=True,
                )

            # Input DMAs on the SP ring.  Separate tiles per W half and per
            # x chunk keep the dependency tracker's intervals precise, so
            # the first matmul of a chunk waits only on its own data.  W is
            # prefetched 3 segments deep, x 2 deep (pool bufs sized so no
            # prefetch DMA ever waits on tile rotation, which would
            # head-of-line block the FIFO ring).
            def load_w(e, trickle=False):
                w_lo = wpool.tile([P, KO, O // 2], dt, tag="wlo", name="wlo")
                w_hi = wpool.tile([P, KO, O // 2], dt, tag="whi", name="whi")
                if trickle:
                    for ko in range(KO):
                        nc.sync.dma_start(
                            out=w_lo[:, ko], in_=w_d[e, 0, :, ko]
                        )
                else:
                    nc.sync.dma_start(out=w_lo[:], in_=w_d[e, 0])
                nc.sync.dma_start(out=w_hi[:], in_=w_d[e, 1])
                return w_lo, w_hi

            def load_x(e, trickle=False):
                base = int(plan.seg_base[e])
                x_cs = []
                for ci, (c0, L) in enumerate(seg_chunks[e]):
                    f0 = KO * (base + c0)
                    xc = xpool.tile([P, KO * L], dt, tag="x", name="xc")
                    if trickle and ci == 0:
                        for ko in range(KO):
                            nc.sync.dma_start(
                                out=xc[:, ko * L : (ko + 1) * L],
                                in_=xg_d[:, f0 + ko * L : f0 + (ko + 1) * L],
                            )
                    else:
                        nc.sync.dma_start(
                            out=xc[:], in_=xg_d[:, f0 : f0 + KO * L]
                        )
                    x_cs.append(xc)
                return x_cs

            # start: segment 0 per-ko trickled (kernel start is HBM-bound
            # with all 8 cores pulling their first supply at once — let the
            # matmuls chase the arrivals), then staggered prefetch.
            base0 = int(plan.seg_base[0])
            (c00, L00) = seg_chunks[0][0]
            w_lo0 = wpool.tile([P, KO, O // 2], dt, tag="wlo", name="wlo")
            w_hi0 = wpool.tile([P, KO, O // 2], dt, tag="whi", name="whi")
            xc00 = xpool.tile([P, KO * L00], dt, tag="x", name="xc")
            f00 = KO * (base0 + c00)
            for ko in range(KO):
                nc.sync.dma_start(out=w_lo0[:, ko], in_=w_d[0, 0, :, ko])
                nc.sync.dma_start(
                    out=xc00[:, ko * L00 : (ko + 1) * L00],
                    in_=xg_d[:, f00 + ko * L00 : f00 + (ko + 1) * L00],
                )
            nc.sync.dma_start(out=w_hi0[:], in_=w_d[0, 1])
            x_cs0 = [xc00]
            for ci, (c0, L) in enumerate(seg_chunks[0][1:]):
                f0 = KO * (base0 + c0)
                xc = xpool.tile([P, KO * L], dt, tag="x", name="xc")
                nc.sync.dma_start(out=xc[:], in_=xg_d[:, f0 : f0 + KO * L])
                x_cs0.append(xc)
            w_bufs = {0: (w_lo0, w_hi0), 1: load_w(1)}
            x_bufs = {0: x_cs0, 1: load_x(1)}
            g_sb = cpool.tile([P, plan.PAIRS], adt)
            # gates for segments 0-1 early (Act ring, small, beats the first
            # eviction); the rest rides the SP ring behind segment-2 inputs
            # to keep the HBM-bound start window lean.
            g_split = int(plan.seg_base[2])
            nc.scalar.dma_start(out=g_sb[:, :g_split], in_=g_d[:, :g_split])
            w_bufs[2] = load_w(2)
            nc.sync.dma_start(out=g_sb[:, g_split:], in_=g_d[:, g_split:])

            def combine(r0, r1):
                """One staging tile + one contiguous-flat DMA per piece."""
                Lp = r1 - r0
                og = ogpool.tile(
                    [P, OB * Lp], mybir.dt.float32, tag="og", name="og"
                )
                view = og[:, : OB * Lp].rearrange("p (a b) -> p a b", a=OB)
                nc.vector.tensor_add(
                    out=view,
                    in0=arena1[:, :, r0:r1],
                    in1=arena2[:, :, r0:r1],
                )
                nc.scalar.dma_start(
                    out=out_d[:, OB * r0 : OB * r1], in_=og[:, : OB * Lp]
                )

            for e in range(E):
                (w_lo, w_hi) = w_bufs.pop(e)
                x_cs = x_bufs.pop(e)
                if e + 2 < E:
                    x_bufs[e + 2] = load_x(e + 2)
                if e + 3 < E:
                    w_bufs[e + 3] = load_w(e + 3)
                base = int(plan.seg_base[e])
                for ci, (c0, L) in enumerate(seg_chunks[e]):
                    for g2 in range(4):
                        w_half = w_lo if g2 < 2 else w_hi
                        ps = pspool.tile(
                            [P, 2, CHUNK], mybir.dt.float32, tag="ps"
                        )
                        for ob2 in range(2):
                            wc = ((g2 % 2) * 2 + ob2) * P
                            for ko in range(KO):
                                nc.tensor.matmul(
                                    out=ps[:, ob2, :L],
                                    lhsT=w_half[:, ko, wc : wc + P],
                                    rhs=x_cs[ci][:, ko * L : (ko + 1) * L],
                                    start=(ko == 0),
                                    stop=(ko == KO - 1),
                                )
                        # evict with per-column gate multiply
                        for (soff, ooff, rl, role) in plan.runs[e]:
                            lo = max(soff, c0)
                            hi = min(soff + rl, c0 + L)
                            if lo >= hi:
                                continue
                            arena = arena1 if role == 1 else arena2
                            o0 = ooff + (lo - soff)
                            nc.vector.tensor_mul(
                                out=arena[
                                    :, g2 * 2 : (g2 + 1) * 2, o0 : o0 + hi - lo
                                ],
                                in0=ps[:, :, lo - c0 : hi - c0],
                                in1=g_sb[
                                    :, None, base + lo : base + hi
                                ].broadcast_to([P, 2, hi - lo]),
                            )
                    # combine the role-2 cols this chunk completed
                    for (pe, pci, r0, r1) in plan.pieces:
                        if pe == e and pci == ci:
                            combine(r0, r1)
    return nc


def kernel(x, gates, W, b):
    _patch_tile_drain()
    dt_name = os.environ.get("MOE_DT", "float16")
    dt = {
        "float16": mybir.dt.float16,
        "bfloat16": mybir.dt.bfloat16,
    }[dt_name]
    np_dt = {"float16": np.float16, "bfloat16": ml_dtypes.bfloat16}[dt_name]
    adt = mybir.dt.float16  # gate / arena dtype

    gates = np.asarray(gates)
    x = np.ascontiguousarray(x)
    W = np.asarray(W)
    b = np.asarray(b)
    assert not np.any(b), "bias path not implemented (reference uses zeros)"

    plan = Plan(gates)
    wb = np.ascontiguousarray(
        W.astype(np_dt).reshape(E, KO, P, 2, O // 2).transpose(0, 3, 2, 1, 4)
    )  # [E, half, ki, ko, o_col] — contiguous per (e, half, ki)
    in_maps = []
    for c in range(N_CORES):
        m = _build_core_inputs(x, gates, plan, c, np_dt)
        m["w"] = wb
        in_maps.append(m)

    nc = _build_program(plan, dt, adt)

    trace = os.environ.get("MOE_TRACE", "0") == "1"
    kwargs = {}
    if trace:
        _install_ntff_shim()
        kwargs = dict(trace=True, trace_cores=list(range(N_CORES)))

    res = bass_utils.run_bass_kernel_spmd(
        nc, in_maps, core_ids=list(range(N_CORES)), **kwargs
    )
    if trace and res.exec_time_ns is not None:
        print(
            f"HW exec time: {res.exec_time_ns} ns "
            f"(mean {res.mean_exec_time_ns:.0f})"
        )

    out = np.empty((B, O), np.float32)
    for c in range(N_CORES):
        co = res.results[c]["out"]  # [P, OB * n_slots], piece-major flat
        arr = np.empty((plan.n_slots, O), np.float32)
        for (_, _, r0, r1) in plan.pieces:
            seg = co[:, OB * r0 : OB * r1].reshape(P, OB, r1 - r0)
            arr[r0:r1] = seg.transpose(2, 1, 0).reshape(r1 - r0, O)
        toks = plan.core_tokens(c)
        for t in range(NT):
            o0 = int(plan.out_off[t])
            out[toks[t]] = arr[o0 : o0 + len(toks[t])]
    return out


def _install_ntff_shim():
    """Best-effort: register the missing antenv.axon_hooks NTFF profile hook
    so trace=True yields exec_time_ns.  Only used when MOE_TRACE=1."""
    try:
        import antenv
        from trn_agent_boot.trn_boot import _ntff_profile_via_ctypes

        if "antenv.axon_hooks" in sys.modules:
            return
        hooks = types.ModuleType("antenv.axon_hooks")
        hook = _ntff_profile_via_ctypes("/opt/axon/libaxon_pjrt.so")
        hooks.get_axon_ntff_profile_hook = lambda: hook
        hooks.set_axon_ntff_profile_hook = lambda h: None
        sys.modules["antenv.axon_hooks"] = hooks
        antenv.axon_hooks = hooks
        bass_utils.upload_artifacts = lambda tmpdir: tmpdir
    except Exception as e:  # pragma: no cover
        print(f"ntff shim unavailable: {e}", file=sys.stderr)
